# revision 30
# baseline (speedup 1.0000x reference)
"""Trainium2 Bass kernel for the JobActor GNN (2-layer GIN + actor MLP + masked softmax).

Sharding: data-parallel over batch B=8 -- one graph per NeuronCore. Params replicated.

I/O shape (host overhead dominates the measured single-shot time on this
axon client: ~25-40 us per operand per call): exactly TWO inputs per core --
adj [4096, 4096] fp32 and ONE packed [128, 872] fp32 tensor carrying every
small input (features/pool/actor_w1 pre-permuted host-side, weights, biases,
candidate/mask as f32). On-chip, everything is an AP view into the packed tile.

Per-core device strategy (memory-bound on adj; fp8 cast stream = 157 us
HW-probed floor; steady-state measured ~188 us):
  - Stream adj from HBM exactly ONCE: 32 x 2 MiB SWDGE DMAs with fp32 -> fp8e4
    cast in flight (exact on adjacency values {0,1,2}; 80 MiB combined traffic
    vs 96 for bf16 -- HW-probed 157 vs 187 us). The gpsimd queue carries ONLY
    this stream: ANY post-stream op on the Pool queue head-of-line blocks the
    next invocation's stream (probed +140 us). Queue-splitting the stream
    (SWDGE+HWDGE mix) was probed SLOWER (209 us) -- DMA-engine contention.
  - Transpose each 128x128 block with a REGULAR matmul against an fp8 identity
    (stationary = fp8 adj block => fast-weight-load). Four blocks share one
    [128, 512] fp32 PSUM tile; one big PSUM->SBUF copy (fp32 -> fp8,
    alternating DVE/ACT; Pool cannot evacuate PSUM -- codegen rejects it)
    lands in the resident fp8 adjT (16 MiB). nat bufs=6 gives the stream DMA
    lookahead so compute hiccups never stall it.
  - Both spmm passes use the S2 orientation: stationary = adjT 128x128 block
    (fp8 fast-weight-load), moving = h natural [128, F] bf16. HW-probed ~3-4x
    faster than moving 512-wide adjT slabs through a bf16 stationary.
  - GIN MLPs run in fp16 throughout (weights, pooled chunks, hidden layer):
    ~0.05% quantization vs bf16's 0.4%, full PE moving rate vs fp32's
    quarter rate (fp32 moving was the dominant PE serialization: 284 -> 181 us
    when switched). PSUM accumulation stays fp32; bf16 hidden was probed at
    1.97e-2 rel err (too close to the 2e-2 gate), fp16 lands at ~5e-3.
  - Pipeline: spmm0 chunk c runs as slabs 4c..4c+3 land; its MLP is deferred
    one slab so the PE FIFO never stalls on ACT/DVE round trips; phase C
    emits spmm1 two chunks ahead of the MLPs for the same reason. The jb<16
    half of spmm1 runs inside the stream (h1 slabs 0..15 are ready after
    super-iteration 3); phase C computes the jb>=16 half and merges on DVE.
  - Candidate gather = one-hot matmul (iota + is_equal, prebuilt rhs columns);
    graph pooling rides column 0 of the same accumulation. Actor MLP fp32;
    masked softmax on-chip (exp + accumulated sum in one ACT op). actor_b3
    shifts all logits equally and cancels in softmax.
"""

import os
from contextlib import ExitStack

import numpy as np

import concourse.bass as bass
import concourse.bacc as bacc
import concourse.tile as tile
from concourse import mybir
from concourse.bass import ts
from concourse.bass_utils import run_bass_kernel_spmd
from concourse.masks import make_identity

B = 8
N = 4096
IN_DIM = 2
HID = 64
J = 128
P = 128          # SBUF partitions
NB = N // P      # 32 node blocks
CH = 512         # free-dim chunk for spmm / MLPs
NCH = N // CH    # 8 chunks
SLABS_PER_CH = CH // P  # 4

FP32 = mybir.dt.float32
BF16 = mybir.dt.bfloat16
FP8 = mybir.dt.float8e4
FP16 = mybir.dt.float16
I32 = mybir.dt.int32

AF = mybir.ActivationFunctionType

LAST_EXEC_NS = None

# ---- packed-small-input column offsets (one [P, PKC] fp32 DRAM tensor) ----
# Host/axon dispatch costs ~25-40 us per operand per call; packing all small
# inputs into one tensor cuts ~17 operands (~0.5 ms measured).
F_OFF = 0            # features  [P, NB*IN_DIM]  (nb-major)    cols 0:64
GP_OFF = 64          # graph_pool [P, NB]                      cols 64:96
AW1_OFF = 96         # actor_w1  [64, 3*64] (mid-major)        cols 96:288
W01_OFF = 288        # gin0_w1 [2,64]
W02_OFF = 352        # gin0_w2 [64,64]
W11_OFF = 416        # gin1_w1
W12_OFF = 480        # gin1_w2
AW2_OFF = 544        # actor_w2
B01_OFF, B02_OFF, B11_OFF, B12_OFF = 608, 609, 610, 611   # [64,1] each
AB1_OFF, AB2_OFF, PMI_OFF, AW3_OFF = 612, 613, 614, 615
CAND_OFF = 616       # candidate as f32, [1,128] on partition 0
MASK_OFF = 744       # mask as f32, [1,128] on partition 0
PKC = 872


def _build_kernel(ctx: ExitStack, tc: tile.TileContext, io: dict):
    nc = tc.nc

    consts = ctx.enter_context(tc.tile_pool(name="consts", bufs=1))
    resident = ctx.enter_context(tc.tile_pool(name="resident", bufs=1))
    nat_pool = ctx.enter_context(tc.tile_pool(name="nat", bufs=6))
    work = ctx.enter_context(tc.tile_pool(name="work", bufs=2))
    psum_tr = ctx.enter_context(tc.tile_pool(name="psum_tr", bufs=3, space="PSUM"))
    psum_acc = ctx.enter_context(tc.tile_pool(name="psum_acc", bufs=3, space="PSUM"))
    psum_sm = ctx.enter_context(tc.tile_pool(name="psum_sm", bufs=2, space="PSUM"))

    # ---------------- constants / params ----------------
    ident16 = consts.tile([P, P], BF16)
    make_identity(nc, ident16)
    ident32 = consts.tile([P, P], FP32)
    make_identity(nc, ident32)
    ident8 = consts.tile([P, P], FP8)
    nc.vector.tensor_copy(out=ident8, in_=ident16)
    ident_h = consts.tile([P, P], FP16)
    nc.vector.tensor_copy(out=ident_h, in_=ident16)

    # Keep the gpsimd (SWDGE) queue exclusively for the adj stream. All small
    # inputs arrive in ONE packed [P, PKC] fp32 tensor on the sync (HWDGE)
    # queue -- layout permutations (features/pool/actor_w1) are done host-side
    # in make_in_maps; on-chip everything is an AP view into the packed tile.
    pk = consts.tile([P, PKC], FP32)
    nc.sync.dma_start(out=pk, in_=io["packed"])

    cand_sb = pk[0:1, CAND_OFF:CAND_OFF + J]
    mask_sb = pk[0:1, MASK_OFF:MASK_OFF + J]
    feat_sb = consts.tile([P, NB * IN_DIM], BF16)
    nc.vector.tensor_copy(out=feat_sb, in_=pk[:, F_OFF:F_OFF + NB * IN_DIM])

    # GIN MLP weights in fp16 (~0.05% quantization; psum accumulation stays
    # fp32). Saves the 4x fp32 moving-rate penalty on the PE.
    w01 = consts.tile([IN_DIM, HID], FP16)
    nc.vector.tensor_copy(out=w01, in_=pk[0:IN_DIM, W01_OFF:W01_OFF + HID])
    w02 = consts.tile([HID, HID], FP16)
    nc.vector.tensor_copy(out=w02, in_=pk[0:HID, W02_OFF:W02_OFF + HID])
    w11 = consts.tile([HID, HID], FP16)
    nc.vector.tensor_copy(out=w11, in_=pk[0:HID, W11_OFF:W11_OFF + HID])
    w12 = consts.tile([HID, HID], FP16)
    nc.vector.tensor_copy(out=w12, in_=pk[0:HID, W12_OFF:W12_OFF + HID])
    b01 = pk[0:HID, B01_OFF:B01_OFF + 1]
    b02 = pk[0:HID, B02_OFF:B02_OFF + 1]
    b11 = pk[0:HID, B11_OFF:B11_OFF + 1]
    b12 = pk[0:HID, B12_OFF:B12_OFF + 1]
    pmi = pk[0:HID, PMI_OFF:PMI_OFF + 1]
    aw2 = pk[0:HID, AW2_OFF:AW2_OFF + HID]
    ab1 = pk[0:HID, AB1_OFF:AB1_OFF + 1]
    ab2 = pk[0:HID, AB2_OFF:AB2_OFF + 1]
    aw3 = pk[0:HID, AW3_OFF:AW3_OFF + 1]

    # Persistent activations
    adjT = resident.tile([P, NB, N], FP8)            # adj.T, resident (16 MiB)
    h1nat = resident.tile([P, NB, HID], BF16)        # h1 natural (spmm1 stationary)
    h2nat = resident.tile([P, NB, HID], BF16)        # h2 natural (readout stationary)
    rhs_all = resident.tile([P, NB, 1 + J], BF16)    # readout [pool | one-hot] cols
    p0nat = resident.tile([P, NB, IN_DIM], FP16)     # pooled0 natural (S2 spmm out)
    p1nat = resident.tile([P, NB, HID], FP32)        # pooled1 natural (S2 spmm out)

    adj = io["adj"]


    def gin_mlp(pXc, w_a, b_a, w_b, b_b, hnat, c):
        """2-layer fp32 ReLU MLP on transposed chunk [*, CH] + store natural h.

        hc is written bf16 straight from PSUM (same value the reference path
        would store after its bf16 hnat copy), so the 4 layout transposes run
        as fast bf16 regular matmuls."""
        psa = psum_acc.tile([HID, CH], FP32, tag="acc")
        nc.tensor.matmul(psa, w_a, pXc)
        ha = work.tile([HID, CH], FP16, tag="ha")
        nc.scalar.activation(ha, psa, AF.Relu, bias=b_a)
        psb = psum_acc.tile([HID, CH], FP32, tag="acc")
        nc.tensor.matmul(psb, w_b, ha)
        hc = work.tile([HID, CH], BF16, tag="hc")
        nc.scalar.activation(hc, psb, AF.Relu, bias=b_b)
        # -> natural layout [node, feat] via 4 small transposing matmuls
        for s in range(SLABS_PER_CH):
            pt = psum_sm.tile([P, HID], FP32, tag="pt")
            nc.tensor.matmul(pt, hc[:, ts(s, P)], ident16[:HID, :HID])
            nc.vector.tensor_copy(out=hnat[:, c * SLABS_PER_CH + s, :], in_=pt)

    # =============== pass A: stream adj once; transpose; GIN layer 0 ===============
    p0c_q = []
    for ib in range(NB):
        nat = nat_pool.tile([P, N], FP8)
        nc.gpsimd.dma_start(out=nat, in_=adj[ts(ib, P), :])  # fp32 -> fp8 cast DMA
        # (exact on {0,1,2}; 80 MiB total traffic vs 96 for bf16 -> ~157 us)
        for jq in range(NB // 4):  # 4 block-transposes -> one [128, 512] psum tile
            ptr = psum_tr.tile([P, 4, P], FP32, tag="tr")
            for k in range(4):
                jb = 4 * jq + k
                # regular matmul: out = nat_blk.T @ I  (fp8 stationary -> FWL)
                nc.tensor.matmul(ptr[:, k, :], nat[:, ts(jb, P)], ident8)
            # one big evacuation, alternating DVE/ACT: fp32 -> fp8
            dst = adjT[:, ts(jq, 4), ts(ib, P)]
            if jq % 2 == 0:
                nc.vector.tensor_copy(out=dst, in_=ptr)
            else:
                nc.scalar.copy(out=dst, in_=ptr)

        if ib % SLABS_PER_CH == 1 and ib > SLABS_PER_CH:
            # deferred MLP for the chunk whose spmm finished last super-iteration
            gin_mlp(p0c_q.pop(0), w01, b01, w02, b02, h1nat, ib // SLABS_PER_CH - 1)
        if ib % SLABS_PER_CH != SLABS_PER_CH - 1:
            continue
        c = ib // SLABS_PER_CH
        # ---- GIN layer 0 spmm, S2 form: stationary = adjT block (fp8, FWL
        # fast-weight-load), moving = feats [128, 2]. ~3x faster per element
        # moved than the S1 form (HW-probed). Output is pooled0 NATURAL.
        for k in range(SLABS_PER_CH):
            iblk = c * SLABS_PER_CH + k
            ps0n = psum_acc.tile([P, IN_DIM], FP32, tag="acc")
            for jb in range(NB):
                nc.tensor.matmul(ps0n, adjT[:, jb, ts(iblk, P)], feat_sb[:, ts(jb, IN_DIM)],
                                 start=(jb == 0), stop=(jb == NB - 1))
            nc.vector.tensor_copy(out=p0nat[:, iblk, :], in_=ps0n)
        # pooled0 natural -> transposed chunk [2, 512] via 4 bf16 matmuls
        tp0 = psum_acc.tile([IN_DIM, CH], FP32, tag="acc")
        for k in range(SLABS_PER_CH):
            nc.tensor.matmul(tp0[:, ts(k, P)], p0nat[:, c * SLABS_PER_CH + k, :],
                             ident_h)
        p0c = work.tile([IN_DIM, CH], FP16, tag="p0c")
        nc.scalar.copy(p0c, tp0)
        p0c_q.append(p0c)

        # ---- hide the first half of GIN layer 1's S2 spmm under the stream:
        # h1 slabs 0..15 are ready after super-iteration 3, so their partial
        # contribution to pooled1 (8 i-blocks per iteration) runs here.
        if c >= SLABS_PER_CH:
            for iblk in range(8 * (c - 4), 8 * (c - 4) + 8):
                psh = psum_acc.tile([P, HID], FP32, tag="acc")
                for jb in range(NB // 2):
                    nc.tensor.matmul(psh, adjT[:, jb, ts(iblk, P)], h1nat[:, jb, :],
                                     start=(jb == 0), stop=(jb == NB // 2 - 1))
                nc.vector.tensor_copy(out=p1nat[:, iblk, :], in_=psh)
            # second-half partials (jb 16..23) once h1 chunks 4-5 exist: 16
            # iblks at each of super-iterations 6 and 7. Leaves only jb>=24
            # for phase C (halves its serial spmm tail).
            if c >= 6:
                for iblk in range(16 * (c - 6), 16 * (c - 6) + 16):
                    psh2 = psum_acc.tile([P, HID], FP32, tag="acc")
                    for jb in range(16, 24):
                        nc.tensor.matmul(psh2, adjT[:, jb, ts(iblk, P)],
                                         h1nat[:, jb, :],
                                         start=(jb == 16), stop=(jb == 23))
                    nc.vector.tensor_add(out=p1nat[:, iblk, :],
                                         in0=p1nat[:, iblk, :], in1=psh2)

    # =============== phase C: GIN layer 1 (spmm from resident adjT) ===============
    # Readout prep first: its DVE/PE ops overlap with spmm1's PE stream.
    # (iota sits on the gpsimd queue AFTER all adj dma_starts -- no stall.)
    iota_i = consts.tile([P, NB], I32)
    nc.gpsimd.iota(iota_i, pattern=[[P, NB]], base=0, channel_multiplier=1)
    iota_f = consts.tile([P, NB], FP32)
    nc.vector.tensor_copy(out=iota_f, in_=iota_i)
    ones1 = consts.tile([1, P], FP32)
    nc.vector.memset(ones1, 1.0)
    ps_cb = psum_sm.tile([P, J], FP32, tag="pt")
    nc.tensor.matmul(ps_cb, ones1, cand_sb)
    cand_bc = consts.tile([P, J], FP32)
    nc.scalar.copy(cand_bc, ps_cb)
    maskneg = consts.tile([1, J], FP32)
    nc.scalar.mul(maskneg, mask_sb, -1e30)
    for jb in range(NB):
        # NOT on Pool: post-stream ops on the Pool queue head-of-line block
        # the next invocation's SWDGE adj stream (probed +140us/rep).
        nc.vector.tensor_copy(out=rhs_all[:, jb, 0:1],
                              in_=pk[:, GP_OFF + jb:GP_OFF + jb + 1])
        nc.vector.tensor_scalar(
            out=rhs_all[:, jb, 1:1 + J], in0=cand_bc, scalar1=iota_f[:, jb:jb + 1],
            scalar2=None, op0=mybir.AluOpType.is_equal)

    # drain the deferred layer-0 MLP for the last chunk
    gin_mlp(p0c_q.pop(0), w01, b01, w02, b02, h1nat, NCH - 1)

    def spmm1(c):
        # S2 form, second half (jb 16..31); the first half accumulated into
        # p1nat during the adj stream. Merge on DVE, then 4 fp32 transposing
        # matmuls build the [64, 512] chunk.
        p1f = work.tile([P, SLABS_PER_CH, HID], FP16, tag="p1f")
        for k in range(SLABS_PER_CH):
            iblk = c * SLABS_PER_CH + k
            ps1n = psum_tr.tile([P, HID], FP32, tag="tr")  # tr pool idle in C
            for jb in range(24, NB):
                nc.tensor.matmul(ps1n, adjT[:, jb, ts(iblk, P)], h1nat[:, jb, :],
                                 start=(jb == 24), stop=(jb == NB - 1))
            nc.vector.tensor_add(out=p1f[:, k, :], in0=ps1n, in1=p1nat[:, iblk, :])
        tp1 = psum_tr.tile([HID, CH], FP32, tag="tr")
        for k in range(SLABS_PER_CH):
            nc.tensor.matmul(tp1[:, ts(k, P)], p1f[:, k, :], ident_h)
        p1c = work.tile([HID, CH], FP16, tag="p1c")
        nc.scalar.copy(p1c, tp1)
        return p1c

    LOOKAHEAD = 2
    p1c_q = [spmm1(c) for c in range(LOOKAHEAD)]
    for c in range(NCH):
        if c + LOOKAHEAD < NCH:
            p1c_q.append(spmm1(c + LOOKAHEAD))
        gin_mlp(p1c_q.pop(0), w11, b11, w12, b12, h2nat, c)

    # =============== phase D: pooling + gather + actor MLP + masked softmax ===============
    # [graph_pool column | one-hot gather matrix] @ h2  -> [g | jobs.T] in one chain
    ps_gj = psum_acc.tile([HID, 1 + J], FP32, tag="acc")
    for jb in range(NB):
        nc.tensor.matmul(ps_gj, h2nat[:, jb, :], rhs_all[:, jb, :],
                         start=(jb == 0), stop=(jb == NB - 1))
    gcol = consts.tile([HID, 1], FP32)
    nc.scalar.copy(gcol, ps_gj[:, 0:1])
    jobsT = consts.tile([HID, J], FP32)
    nc.scalar.copy(jobsT, ps_gj[:, 1:1 + J])

    # combined per-partition bias: W1b.T @ g + W1c.T @ pmi + actor_b1
    ps_bc = psum_acc.tile([HID, 1], FP32, tag="acc")
    nc.tensor.matmul(ps_bc, pk[0:HID, AW1_OFF + HID:AW1_OFF + 2 * HID], gcol,
                     start=True, stop=False)
    nc.tensor.matmul(ps_bc, pk[0:HID, AW1_OFF + 2 * HID:AW1_OFF + 3 * HID], pmi,
                     start=False, stop=True)
    bias_c = consts.tile([HID, 1], FP32)
    nc.scalar.copy(bias_c, ps_bc)
    bias_tot = consts.tile([HID, 1], FP32)
    nc.vector.tensor_add(out=bias_tot, in0=bias_c, in1=ab1)

    ps_a1 = psum_acc.tile([HID, J], FP32, tag="acc")
    nc.tensor.matmul(ps_a1, pk[0:HID, AW1_OFF:AW1_OFF + HID], jobsT)
    a1 = consts.tile([HID, J], FP32)
    nc.scalar.activation(a1, ps_a1, AF.Tanh, bias=bias_tot)
    ps_a2 = psum_acc.tile([HID, J], FP32, tag="acc")
    nc.tensor.matmul(ps_a2, aw2, a1)
    a2 = consts.tile([HID, J], FP32)
    nc.scalar.activation(a2, ps_a2, AF.Tanh, bias=ab2)
    ps_s = psum_acc.tile([1, J], FP32, tag="acc")
    nc.tensor.matmul(ps_s, aw3, a2)
    scores = consts.tile([1, J], FP32)
    nc.scalar.mul(scores, ps_s, 10.0)  # actor_b3 cancels in softmax

    smask = consts.tile([1, J], FP32)
    nc.vector.tensor_add(out=smask, in0=scores, in1=maskneg)
    mmax = consts.tile([1, 1], FP32)
    nc.vector.reduce_max(mmax, smask, axis=mybir.AxisListType.X)
    negm = consts.tile([1, 1], FP32)
    nc.scalar.mul(negm, mmax, -1.0)
    expv = consts.tile([1, J], FP32)
    ssum = consts.tile([1, 1], FP32)
    nc.scalar.activation(expv, smask, AF.Exp, bias=negm, accum_out=ssum)
    rinv = consts.tile([1, 1], FP32)
    nc.vector.reciprocal(rinv, ssum)
    probs = consts.tile([1, J], FP32)
    nc.vector.tensor_scalar_mul(probs, expv, rinv)
    nc.sync.dma_start(out=io["probs"], in_=probs)


_NC_CACHE = {}


def build_nc(reps: int = 1):
    key = ("nc", reps)
    if key in _NC_CACHE:
        return _NC_CACHE[key]
    nc = bacc.Bacc("TRN2", target_bir_lowering=False, debug=False)
    io = {
        "adj": nc.dram_tensor("adj", [N, N], FP32, kind="ExternalInput").ap(),
        "packed": nc.dram_tensor("packed", [P, PKC], FP32, kind="ExternalInput").ap(),
        "probs": nc.dram_tensor("probs", [1, J], FP32, kind="ExternalOutput").ap(),
    }
    with tile.TileContext(nc) as tc:
        for _ in range(reps):
            with ExitStack() as ctx:
                _build_kernel(ctx, tc, io)
    nc.compile()  # bacc legalization: wait-splitting (1 wait/inst on TRN2), DCE, etc.
    _NC_CACHE[key] = nc
    return nc


def _make_packed(inputs, b):
    pkd = np.zeros((P, PKC), np.float32)
    feat = np.asarray(inputs["features"][b], dtype=np.float32).reshape(NB, P, IN_DIM)
    pkd[:, F_OFF:F_OFF + NB * IN_DIM] = feat.transpose(1, 0, 2).reshape(P, NB * IN_DIM)
    gp = np.asarray(inputs["graph_pool"][b], dtype=np.float32).reshape(NB, P)
    pkd[:, GP_OFF:GP_OFF + NB] = gp.T
    aw1 = np.asarray(inputs["actor_w1"], dtype=np.float32).reshape(3, HID, HID)
    pkd[0:HID, AW1_OFF:AW1_OFF + 3 * HID] = aw1.transpose(1, 0, 2).reshape(HID, 3 * HID)
    pkd[0:IN_DIM, W01_OFF:W01_OFF + HID] = np.asarray(inputs["gin0_w1"], np.float32)
    pkd[0:HID, W02_OFF:W02_OFF + HID] = np.asarray(inputs["gin0_w2"], np.float32)
    pkd[0:HID, W11_OFF:W11_OFF + HID] = np.asarray(inputs["gin1_w1"], np.float32)
    pkd[0:HID, W12_OFF:W12_OFF + HID] = np.asarray(inputs["gin1_w2"], np.float32)
    pkd[0:HID, AW2_OFF:AW2_OFF + HID] = np.asarray(inputs["actor_w2"], np.float32)
    for off, name in ((B01_OFF, "gin0_b1"), (B02_OFF, "gin0_b2"),
                      (B11_OFF, "gin1_b1"), (B12_OFF, "gin1_b2"),
                      (AB1_OFF, "actor_b1"), (AB2_OFF, "actor_b2"),
                      (PMI_OFF, "pooled_machine_input"), (AW3_OFF, "actor_w3")):
        pkd[0:HID, off] = np.asarray(inputs[name], np.float32).reshape(HID)
    pkd[0, CAND_OFF:CAND_OFF + J] = np.asarray(inputs["candidate"][b]).astype(np.float32)
    pkd[0, MASK_OFF:MASK_OFF + J] = np.asarray(inputs["mask"][b]).astype(np.float32).reshape(J)
    return pkd


def make_in_maps(inputs):
    return [{
        "adj": np.ascontiguousarray(inputs["adj"][b], dtype=np.float32),
        "packed": _make_packed(inputs, b),
    } for b in range(B)]


def kernel(**inputs) -> np.ndarray:
    global LAST_EXEC_NS
    nc = build_nc()
    in_maps = make_in_maps(inputs)
    # NTFF tracing is unavailable on this axon client (no antenv.axon_hooks);
    # always run untraced. Timing is done separately (see test.py).
    os.environ["BASS_NEVER_TRACE"] = "1"
    res = run_bass_kernel_spmd(nc, in_maps, core_ids=list(range(B)), trace=False)
    LAST_EXEC_NS = res.exec_time_ns
    out = np.stack([np.asarray(res.results[b]["probs"]).reshape(J) for b in range(B)], axis=0)
    return out.astype(np.float32)



# revision 33
# speedup vs baseline: 1.0410x; 1.0410x over previous
"""Trainium2 Bass kernel for the JobActor GNN (2-layer GIN + actor MLP + masked softmax).

Sharding: data-parallel over batch B=8 -- one graph per NeuronCore. Params replicated.

I/O shape (host overhead dominates the measured single-shot time on this
axon client: ~25-40 us per operand per call): exactly TWO inputs per core --
adj [4096, 4096] fp32 and ONE packed [128, 872] fp32 tensor carrying every
small input (features/pool/actor_w1 pre-permuted host-side, weights, biases,
candidate/mask as f32). On-chip, everything is an AP view into the packed tile.

Per-core device strategy (memory-bound on adj; fp8 cast stream = 157 us
HW-probed floor; steady-state measured ~188 us):
  - Stream adj from HBM exactly ONCE: 32 x 2 MiB SWDGE DMAs with fp32 -> fp8e4
    cast in flight (exact on adjacency values {0,1,2}; 80 MiB combined traffic
    vs 96 for bf16 -- HW-probed 157 vs 187 us). The gpsimd queue carries ONLY
    this stream: ANY post-stream op on the Pool queue head-of-line blocks the
    next invocation's stream (probed +140 us). Queue-splitting the stream
    (SWDGE+HWDGE mix) was probed SLOWER (209 us) -- DMA-engine contention.
  - Transpose each 128x128 block with a REGULAR matmul against an fp8 identity
    (stationary = fp8 adj block => fast-weight-load). Four blocks share one
    [128, 512] fp32 PSUM tile; one big PSUM->SBUF copy (fp32 -> fp8,
    alternating DVE/ACT; Pool cannot evacuate PSUM -- codegen rejects it)
    lands in the resident fp8 adjT (16 MiB). nat bufs=6 gives the stream DMA
    lookahead so compute hiccups never stall it.
  - Both spmm passes use the S2 orientation: stationary = adjT 128x128 block
    (fp8 fast-weight-load), moving = h natural [128, F] bf16. HW-probed ~3-4x
    faster than moving 512-wide adjT slabs through a bf16 stationary.
  - GIN MLPs run in fp16 throughout (weights, pooled chunks, hidden layer):
    ~0.05% quantization vs bf16's 0.4%, full PE moving rate vs fp32's
    quarter rate (fp32 moving was the dominant PE serialization: 284 -> 181 us
    when switched). PSUM accumulation stays fp32; bf16 hidden was probed at
    1.97e-2 rel err (too close to the 2e-2 gate), fp16 lands at ~5e-3.
  - Pipeline: spmm0 chunk c runs as slabs 4c..4c+3 land; its MLP is deferred
    one slab so the PE FIFO never stalls on ACT/DVE round trips; phase C
    emits spmm1 two chunks ahead of the MLPs for the same reason. The jb<16
    half of spmm1 runs inside the stream (h1 slabs 0..15 are ready after
    super-iteration 3); phase C computes the jb>=16 half and merges on DVE.
  - Candidate gather = one-hot matmul (iota + is_equal, prebuilt rhs columns);
    graph pooling rides column 0 of the same accumulation. Actor MLP fp32;
    masked softmax on-chip (exp + accumulated sum in one ACT op). actor_b3
    shifts all logits equally and cancels in softmax.
"""

import os
from contextlib import ExitStack

import numpy as np

import concourse.bass as bass
import concourse.bacc as bacc
import concourse.tile as tile
from concourse import mybir
from concourse.bass import ts
from concourse.bass_utils import run_bass_kernel_spmd
from concourse.masks import make_identity

B = 8
N = 4096
IN_DIM = 2
HID = 64
J = 128
P = 128          # SBUF partitions
NB = N // P      # 32 node blocks
CH = 512         # free-dim chunk for spmm / MLPs
NCH = N // CH    # 8 chunks
SLABS_PER_CH = CH // P  # 4

FP32 = mybir.dt.float32
BF16 = mybir.dt.bfloat16
FP8 = mybir.dt.float8e4
FP16 = mybir.dt.float16
I32 = mybir.dt.int32

AF = mybir.ActivationFunctionType

LAST_EXEC_NS = None

# ---- packed-small-input column offsets (one [P, PKC] fp32 DRAM tensor) ----
# Host/axon dispatch costs ~25-40 us per operand per call; packing all small
# inputs into one tensor cuts ~17 operands (~0.5 ms measured).
F_OFF = 0            # features  [P, NB*IN_DIM]  (nb-major)    cols 0:64
GP_OFF = 64          # graph_pool [P, NB]                      cols 64:96
AW1_OFF = 96         # actor_w1  [64, 3*64] (mid-major)        cols 96:288
W01_OFF = 288        # gin0_w1 [2,64]
W02_OFF = 352        # gin0_w2 [64,64]
W11_OFF = 416        # gin1_w1
W12_OFF = 480        # gin1_w2
AW2_OFF = 544        # actor_w2
B01_OFF, B02_OFF, B11_OFF, B12_OFF = 608, 609, 610, 611   # [64,1] each
AB1_OFF, AB2_OFF, PMI_OFF, AW3_OFF = 612, 613, 614, 615
CAND_OFF = 616       # candidate as f32, [1,128] on partition 0
MASK_OFF = 744       # mask as f32, [1,128] on partition 0
PKC = 872


def _build_kernel(ctx: ExitStack, tc: tile.TileContext, io: dict):
    nc = tc.nc

    consts = ctx.enter_context(tc.tile_pool(name="consts", bufs=1))
    resident = ctx.enter_context(tc.tile_pool(name="resident", bufs=1))
    nat_pool = ctx.enter_context(tc.tile_pool(name="nat", bufs=6))
    work = ctx.enter_context(tc.tile_pool(name="work", bufs=2))
    psum_tr = ctx.enter_context(tc.tile_pool(name="psum_tr", bufs=3, space="PSUM"))
    psum_acc = ctx.enter_context(tc.tile_pool(name="psum_acc", bufs=3, space="PSUM"))
    psum_sm = ctx.enter_context(tc.tile_pool(name="psum_sm", bufs=2, space="PSUM"))

    # ---------------- constants / params ----------------
    ident16 = consts.tile([P, P], BF16)
    make_identity(nc, ident16)
    ident32 = consts.tile([P, P], FP32)
    make_identity(nc, ident32)
    ident8 = consts.tile([P, P], FP8)
    nc.vector.tensor_copy(out=ident8, in_=ident16)
    ident_h = consts.tile([P, P], FP16)
    nc.vector.tensor_copy(out=ident_h, in_=ident16)

    # Keep the gpsimd (SWDGE) queue exclusively for the adj stream. All small
    # inputs arrive in ONE packed [P, PKC] fp32 tensor on the sync (HWDGE)
    # queue -- layout permutations (features/pool/actor_w1) are done host-side
    # in make_in_maps; on-chip everything is an AP view into the packed tile.
    pk = consts.tile([P, PKC], FP32)
    nc.sync.dma_start(out=pk, in_=io["packed"])

    cand_sb = pk[0:1, CAND_OFF:CAND_OFF + J]
    mask_sb = pk[0:1, MASK_OFF:MASK_OFF + J]
    feat_sb = consts.tile([P, NB * IN_DIM], BF16)
    nc.vector.tensor_copy(out=feat_sb, in_=pk[:, F_OFF:F_OFF + NB * IN_DIM])

    # GIN MLP weights in fp16 (~0.05% quantization; psum accumulation stays
    # fp32). Saves the 4x fp32 moving-rate penalty on the PE.
    w01 = consts.tile([IN_DIM, HID], FP16)
    nc.vector.tensor_copy(out=w01, in_=pk[0:IN_DIM, W01_OFF:W01_OFF + HID])
    w02 = consts.tile([HID, HID], FP16)
    nc.vector.tensor_copy(out=w02, in_=pk[0:HID, W02_OFF:W02_OFF + HID])
    w11 = consts.tile([HID, HID], FP16)
    nc.vector.tensor_copy(out=w11, in_=pk[0:HID, W11_OFF:W11_OFF + HID])
    w12 = consts.tile([HID, HID], FP16)
    nc.vector.tensor_copy(out=w12, in_=pk[0:HID, W12_OFF:W12_OFF + HID])
    b01 = pk[0:HID, B01_OFF:B01_OFF + 1]
    b02 = pk[0:HID, B02_OFF:B02_OFF + 1]
    b11 = pk[0:HID, B11_OFF:B11_OFF + 1]
    b12 = pk[0:HID, B12_OFF:B12_OFF + 1]
    pmi = pk[0:HID, PMI_OFF:PMI_OFF + 1]
    aw2 = pk[0:HID, AW2_OFF:AW2_OFF + HID]
    ab1 = pk[0:HID, AB1_OFF:AB1_OFF + 1]
    ab2 = pk[0:HID, AB2_OFF:AB2_OFF + 1]
    aw3 = pk[0:HID, AW3_OFF:AW3_OFF + 1]

    # Persistent activations
    adjT = resident.tile([P, NB, N], FP8)            # adj.T, resident (16 MiB)
    h1nat = resident.tile([P, NB, HID], BF16)        # h1 natural (spmm1 stationary)
    h2nat = resident.tile([P, NB, HID], BF16)        # h2 natural (readout stationary)
    rhs_all = resident.tile([P, NB, 1 + J], BF16)    # readout [pool | one-hot] cols
    p0nat = resident.tile([P, NB, IN_DIM], FP16)     # pooled0 natural (S2 spmm out)
    p1nat = resident.tile([P, NB, HID], FP32)        # pooled1 natural (S2 spmm out)

    adj = io["adj"]


    def gin_mlp(pXc, w_a, b_a, w_b, b_b, hnat, c):
        """2-layer fp32 ReLU MLP on transposed chunk [*, CH] + store natural h.

        hc is written bf16 straight from PSUM (same value the reference path
        would store after its bf16 hnat copy), so the 4 layout transposes run
        as fast bf16 regular matmuls."""
        psa = psum_acc.tile([HID, CH], FP32, tag="acc")
        nc.tensor.matmul(psa, w_a, pXc)
        ha = work.tile([HID, CH], FP16, tag="ha")
        nc.scalar.activation(ha, psa, AF.Relu, bias=b_a)
        psb = psum_acc.tile([HID, CH], FP32, tag="acc")
        nc.tensor.matmul(psb, w_b, ha)
        hc = work.tile([HID, CH], BF16, tag="hc")
        nc.scalar.activation(hc, psb, AF.Relu, bias=b_b)
        # -> natural layout [node, feat] via 4 small transposing matmuls
        for s in range(SLABS_PER_CH):
            pt = psum_sm.tile([P, HID], FP32, tag="pt")
            nc.tensor.matmul(pt, hc[:, ts(s, P)], ident16[:HID, :HID])
            nc.vector.tensor_copy(out=hnat[:, c * SLABS_PER_CH + s, :], in_=pt)

    # =============== pass A: stream adj once; transpose; GIN layer 0 ===============
    p0c_q = []
    for ib in range(NB):
        nat = nat_pool.tile([P, N], FP8)
        nc.gpsimd.dma_start(out=nat, in_=adj[ts(ib, P), :])  # fp32 -> fp8 cast DMA
        # (exact on {0,1,2}; 80 MiB total traffic vs 96 for bf16 -> ~157 us)
        for jq in range(NB // 4):  # 4 block-transposes -> one [128, 512] psum tile
            ptr = psum_tr.tile([P, 4, P], FP32, tag="tr")
            for k in range(4):
                jb = 4 * jq + k
                # regular matmul: out = nat_blk.T @ I  (fp8 stationary -> FWL)
                nc.tensor.matmul(ptr[:, k, :], nat[:, ts(jb, P)], ident8)
            # one big evacuation, alternating DVE/ACT: fp32 -> fp8
            dst = adjT[:, ts(jq, 4), ts(ib, P)]
            if jq % 2 == 0:
                nc.vector.tensor_copy(out=dst, in_=ptr)
            else:
                nc.scalar.copy(out=dst, in_=ptr)

        if ib % SLABS_PER_CH == 1 and ib > SLABS_PER_CH:
            # deferred MLP for the chunk whose spmm finished last super-iteration
            gin_mlp(p0c_q.pop(0), w01, b01, w02, b02, h1nat, ib // SLABS_PER_CH - 1)
        if ib % SLABS_PER_CH != SLABS_PER_CH - 1:
            continue
        c = ib // SLABS_PER_CH
        # ---- GIN layer 0 spmm, S2 form: stationary = adjT block (fp8, FWL
        # fast-weight-load), moving = feats [128, 2]. ~3x faster per element
        # moved than the S1 form (HW-probed). Output is pooled0 NATURAL.
        for k in range(SLABS_PER_CH):
            iblk = c * SLABS_PER_CH + k
            ps0n = psum_acc.tile([P, IN_DIM], FP32, tag="acc")
            for jb in range(NB):
                nc.tensor.matmul(ps0n, adjT[:, jb, ts(iblk, P)], feat_sb[:, ts(jb, IN_DIM)],
                                 start=(jb == 0), stop=(jb == NB - 1))
            nc.vector.tensor_copy(out=p0nat[:, iblk, :], in_=ps0n)
        # pooled0 natural -> transposed chunk [2, 512] via 4 bf16 matmuls
        tp0 = psum_acc.tile([IN_DIM, CH], FP32, tag="acc")
        for k in range(SLABS_PER_CH):
            nc.tensor.matmul(tp0[:, ts(k, P)], p0nat[:, c * SLABS_PER_CH + k, :],
                             ident_h)
        p0c = work.tile([IN_DIM, CH], FP16, tag="p0c")
        nc.scalar.copy(p0c, tp0)
        p0c_q.append(p0c)

        # ---- hide the first half of GIN layer 1's S2 spmm under the stream:
        # h1 slabs 0..15 are ready after super-iteration 3, so their partial
        # contribution to pooled1 (8 i-blocks per iteration) runs here.
        if c >= SLABS_PER_CH:
            for iblk in range(8 * (c - 4), 8 * (c - 4) + 8):
                psh = psum_acc.tile([P, HID], FP32, tag="acc")
                for jb in range(NB // 2):
                    nc.tensor.matmul(psh, adjT[:, jb, ts(iblk, P)], h1nat[:, jb, :],
                                     start=(jb == 0), stop=(jb == NB // 2 - 1))
                nc.vector.tensor_copy(out=p1nat[:, iblk, :], in_=psh)
            # NOTE: moving spmm1's jb 16..23 partials into super-iters 6-7
            # (in-place p1nat += psum adds) was probed NEUTRAL-to-WORSE
            # (192 vs 173 us median) -- the read-modify-write on the resident
            # p1nat serializes against the phase-C merges. Keep the simple
            # jb<16 / jb>=16 split.

    # =============== phase C: GIN layer 1 (spmm from resident adjT) ===============
    # Readout prep first: its DVE/PE ops overlap with spmm1's PE stream.
    # (iota sits on the gpsimd queue AFTER all adj dma_starts -- no stall.)
    iota_i = consts.tile([P, NB], I32)
    nc.gpsimd.iota(iota_i, pattern=[[P, NB]], base=0, channel_multiplier=1)
    iota_f = consts.tile([P, NB], FP32)
    nc.vector.tensor_copy(out=iota_f, in_=iota_i)
    ones1 = consts.tile([1, P], FP32)
    nc.vector.memset(ones1, 1.0)
    ps_cb = psum_sm.tile([P, J], FP32, tag="pt")
    nc.tensor.matmul(ps_cb, ones1, cand_sb)
    cand_bc = consts.tile([P, J], FP32)
    nc.scalar.copy(cand_bc, ps_cb)
    maskneg = consts.tile([1, J], FP32)
    nc.scalar.mul(maskneg, mask_sb, -1e30)
    for jb in range(NB):
        # NOT on Pool: post-stream ops on the Pool queue head-of-line block
        # the next invocation's SWDGE adj stream (probed +140us/rep).
        nc.vector.tensor_copy(out=rhs_all[:, jb, 0:1],
                              in_=pk[:, GP_OFF + jb:GP_OFF + jb + 1])
        nc.vector.tensor_scalar(
            out=rhs_all[:, jb, 1:1 + J], in0=cand_bc, scalar1=iota_f[:, jb:jb + 1],
            scalar2=None, op0=mybir.AluOpType.is_equal)

    # drain the deferred layer-0 MLP for the last chunk
    gin_mlp(p0c_q.pop(0), w01, b01, w02, b02, h1nat, NCH - 1)

    def spmm1(c):
        # S2 form, second half (jb 16..31); the first half accumulated into
        # p1nat during the adj stream. Merge on DVE, then 4 fp32 transposing
        # matmuls build the [64, 512] chunk.
        p1f = work.tile([P, SLABS_PER_CH, HID], FP16, tag="p1f")
        for k in range(SLABS_PER_CH):
            iblk = c * SLABS_PER_CH + k
            ps1n = psum_tr.tile([P, HID], FP32, tag="tr")  # tr pool idle in C
            for jb in range(NB // 2, NB):
                nc.tensor.matmul(ps1n, adjT[:, jb, ts(iblk, P)], h1nat[:, jb, :],
                                 start=(jb == NB // 2), stop=(jb == NB - 1))
            nc.vector.tensor_add(out=p1f[:, k, :], in0=ps1n, in1=p1nat[:, iblk, :])
        tp1 = psum_tr.tile([HID, CH], FP32, tag="tr")
        for k in range(SLABS_PER_CH):
            nc.tensor.matmul(tp1[:, ts(k, P)], p1f[:, k, :], ident_h)
        p1c = work.tile([HID, CH], FP16, tag="p1c")
        nc.scalar.copy(p1c, tp1)
        return p1c

    LOOKAHEAD = 2
    p1c_q = [spmm1(c) for c in range(LOOKAHEAD)]
    for c in range(NCH):
        if c + LOOKAHEAD < NCH:
            p1c_q.append(spmm1(c + LOOKAHEAD))
        gin_mlp(p1c_q.pop(0), w11, b11, w12, b12, h2nat, c)

    # =============== phase D: pooling + gather + actor MLP + masked softmax ===============
    # [graph_pool column | one-hot gather matrix] @ h2  -> [g | jobs.T] in one chain
    ps_gj = psum_acc.tile([HID, 1 + J], FP32, tag="acc")
    for jb in range(NB):
        nc.tensor.matmul(ps_gj, h2nat[:, jb, :], rhs_all[:, jb, :],
                         start=(jb == 0), stop=(jb == NB - 1))
    gcol = consts.tile([HID, 1], FP32)
    nc.scalar.copy(gcol, ps_gj[:, 0:1])
    jobsT = consts.tile([HID, J], FP32)
    nc.scalar.copy(jobsT, ps_gj[:, 1:1 + J])

    # combined per-partition bias: W1b.T @ g + W1c.T @ pmi + actor_b1
    ps_bc = psum_acc.tile([HID, 1], FP32, tag="acc")
    nc.tensor.matmul(ps_bc, pk[0:HID, AW1_OFF + HID:AW1_OFF + 2 * HID], gcol,
                     start=True, stop=False)
    nc.tensor.matmul(ps_bc, pk[0:HID, AW1_OFF + 2 * HID:AW1_OFF + 3 * HID], pmi,
                     start=False, stop=True)
    bias_c = consts.tile([HID, 1], FP32)
    nc.scalar.copy(bias_c, ps_bc)
    bias_tot = consts.tile([HID, 1], FP32)
    nc.vector.tensor_add(out=bias_tot, in0=bias_c, in1=ab1)

    ps_a1 = psum_acc.tile([HID, J], FP32, tag="acc")
    nc.tensor.matmul(ps_a1, pk[0:HID, AW1_OFF:AW1_OFF + HID], jobsT)
    a1 = consts.tile([HID, J], FP32)
    nc.scalar.activation(a1, ps_a1, AF.Tanh, bias=bias_tot)
    ps_a2 = psum_acc.tile([HID, J], FP32, tag="acc")
    nc.tensor.matmul(ps_a2, aw2, a1)
    a2 = consts.tile([HID, J], FP32)
    nc.scalar.activation(a2, ps_a2, AF.Tanh, bias=ab2)
    ps_s = psum_acc.tile([1, J], FP32, tag="acc")
    nc.tensor.matmul(ps_s, aw3, a2)
    scores = consts.tile([1, J], FP32)
    nc.scalar.mul(scores, ps_s, 10.0)  # actor_b3 cancels in softmax

    smask = consts.tile([1, J], FP32)
    nc.vector.tensor_add(out=smask, in0=scores, in1=maskneg)
    mmax = consts.tile([1, 1], FP32)
    nc.vector.reduce_max(mmax, smask, axis=mybir.AxisListType.X)
    negm = consts.tile([1, 1], FP32)
    nc.scalar.mul(negm, mmax, -1.0)
    expv = consts.tile([1, J], FP32)
    ssum = consts.tile([1, 1], FP32)
    nc.scalar.activation(expv, smask, AF.Exp, bias=negm, accum_out=ssum)
    rinv = consts.tile([1, 1], FP32)
    nc.vector.reciprocal(rinv, ssum)
    probs = consts.tile([1, J], FP32)
    nc.vector.tensor_scalar_mul(probs, expv, rinv)
    nc.sync.dma_start(out=io["probs"], in_=probs)


_NC_CACHE = {}


def build_nc(reps: int = 1):
    key = ("nc", reps)
    if key in _NC_CACHE:
        return _NC_CACHE[key]
    nc = bacc.Bacc("TRN2", target_bir_lowering=False, debug=False)
    io = {
        "adj": nc.dram_tensor("adj", [N, N], FP32, kind="ExternalInput").ap(),
        "packed": nc.dram_tensor("packed", [P, PKC], FP32, kind="ExternalInput").ap(),
        "probs": nc.dram_tensor("probs", [1, J], FP32, kind="ExternalOutput").ap(),
    }
    with tile.TileContext(nc) as tc:
        for _ in range(reps):
            with ExitStack() as ctx:
                _build_kernel(ctx, tc, io)
    nc.compile()  # bacc legalization: wait-splitting (1 wait/inst on TRN2), DCE, etc.
    _NC_CACHE[key] = nc
    return nc


def _make_packed(inputs, b):
    pkd = np.zeros((P, PKC), np.float32)
    feat = np.asarray(inputs["features"][b], dtype=np.float32).reshape(NB, P, IN_DIM)
    pkd[:, F_OFF:F_OFF + NB * IN_DIM] = feat.transpose(1, 0, 2).reshape(P, NB * IN_DIM)
    gp = np.asarray(inputs["graph_pool"][b], dtype=np.float32).reshape(NB, P)
    pkd[:, GP_OFF:GP_OFF + NB] = gp.T
    aw1 = np.asarray(inputs["actor_w1"], dtype=np.float32).reshape(3, HID, HID)
    pkd[0:HID, AW1_OFF:AW1_OFF + 3 * HID] = aw1.transpose(1, 0, 2).reshape(HID, 3 * HID)
    pkd[0:IN_DIM, W01_OFF:W01_OFF + HID] = np.asarray(inputs["gin0_w1"], np.float32)
    pkd[0:HID, W02_OFF:W02_OFF + HID] = np.asarray(inputs["gin0_w2"], np.float32)
    pkd[0:HID, W11_OFF:W11_OFF + HID] = np.asarray(inputs["gin1_w1"], np.float32)
    pkd[0:HID, W12_OFF:W12_OFF + HID] = np.asarray(inputs["gin1_w2"], np.float32)
    pkd[0:HID, AW2_OFF:AW2_OFF + HID] = np.asarray(inputs["actor_w2"], np.float32)
    for off, name in ((B01_OFF, "gin0_b1"), (B02_OFF, "gin0_b2"),
                      (B11_OFF, "gin1_b1"), (B12_OFF, "gin1_b2"),
                      (AB1_OFF, "actor_b1"), (AB2_OFF, "actor_b2"),
                      (PMI_OFF, "pooled_machine_input"), (AW3_OFF, "actor_w3")):
        pkd[0:HID, off] = np.asarray(inputs[name], np.float32).reshape(HID)
    pkd[0, CAND_OFF:CAND_OFF + J] = np.asarray(inputs["candidate"][b]).astype(np.float32)
    pkd[0, MASK_OFF:MASK_OFF + J] = np.asarray(inputs["mask"][b]).astype(np.float32).reshape(J)
    return pkd


def make_in_maps(inputs):
    return [{
        "adj": np.ascontiguousarray(inputs["adj"][b], dtype=np.float32),
        "packed": _make_packed(inputs, b),
    } for b in range(B)]


def kernel(**inputs) -> np.ndarray:
    global LAST_EXEC_NS
    nc = build_nc()
    in_maps = make_in_maps(inputs)
    # NTFF tracing is unavailable on this axon client (no antenv.axon_hooks);
    # always run untraced. Timing is done separately (see test.py).
    os.environ["BASS_NEVER_TRACE"] = "1"
    res = run_bass_kernel_spmd(nc, in_maps, core_ids=list(range(B)), trace=False)
    LAST_EXEC_NS = res.exec_time_ns
    out = np.stack([np.asarray(res.results[b]["probs"]).reshape(J) for b in range(B)], axis=0)
    return out.astype(np.float32)

